# revision 20
# baseline (speedup 1.0000x reference)
"""GRU sequence kernel (Cell2SeqWrapper) for Trainium2, 8 NeuronCores.

Problem: x:(8,4096,256) f32, h0:(8,512), W_ih:(1536,256), W_hh:(1536,512),
b_ih/b_hh:(1536,). Returns (ys:(8,4096,512), h_last:(8,512)) matching

    gx = einsum('btd,gd->btg', x, W_ih) + b_ih
    h' = (1-z) * n + z * h   (standard GRU cell, scan over T)

Sharding: data-parallel over batch: core b processes sequence b end-to-end
(weights replicated). Inside each core:
  - input projection gx computed on-device per 64-step window (TensorE)
  - the serial scan runs one GRU step at a time:
      * gh = W_hh @ h as 12x4 accumulated matvecs, W_hh stationary (bf16,
        fast-weight-load), h moving (bf16 cast of fp32 state)
      * gates on VectorE/ScalarE in fp32, state kept fp32
  - ys accumulated in SBUF, transposed via TensorE, DMA'd out per window
"""

import os
import sys

import numpy as np

sys.path.insert(0, "/opt/trn_rl_repo")

import ml_dtypes  # noqa: E402

import concourse.bass as bass  # noqa: E402
import concourse.mybir as mybir  # noqa: E402
import concourse.tile as tile  # noqa: E402
import concourse.tile_sem_assignment as _tsa  # noqa: E402
from concourse import bass_utils  # noqa: E402
from concourse.bass import ds  # noqa: E402

# Confine HW-DGE DMA completions to a single semaphore lane. With the
# default 8-lane round-robin, the For_i back-edge drain accumulates a
# sync-wait per lane plus one per engine and exceeds the wait-command
# limit in walrus codegen ("Too many sync wait commands").
# (8-lane HW-DGE round-robin kept; oversized wait lists are legalized below)

B, T, D, H = 8, 4096, 256, 512
G = 3 * H            # 1536 gate rows
P = 128              # partitions
KC = H // P          # 4 contraction chunks for W_hh
MC = G // P          # 12 output chunks
DC = D // P          # 2 contraction chunks for W_ih
HC = H // P          # 4 h chunks
W = 128              # scan window (steps per For_i iteration)
NW = T // W

F32 = mybir.dt.float32
BF16 = mybir.dt.bfloat16

# matvec dtype: bf16 (FWL, ~2x faster weight loads) unless overridden
MATVEC_FP32 = bool(int(os.environ.get("BASS_GRU_FP32", "0")))
MV_DT = F32 if MATVEC_FP32 else BF16
MV_NP = np.float32 if MATVEC_FP32 else ml_dtypes.bfloat16

_prog_cache = {}

# walrus's setupSyncWait rejects instructions carrying more than 3 sync
# waits ("Too many sync wait commands"). Tile occasionally emits more
# (loop back-edge drains, staggered-reset stage entries). Legalize by
# moving excess waits onto same-engine NOPs inserted just before the
# instruction — engines execute their queue in order, so the waits
# still complete before the original instruction runs.
_MAX_WAITS = 1


_MAX_UPD = 63


def _legalize_waits(nc):
    counter = [0]

    def mk_nop(engine, on_wait, on_update):
        counter[0] += 1
        nop = mybir.InstNoOp(name=f"I-legalsync-{counter[0]}", ins=[], outs=[])
        nop.engine = engine
        nop.sync_info = mybir.SyncInfo(on_wait=on_wait, on_update=on_update)
        nc.register_instruction(nop, overwrite=True)
        return nop

    for fn in nc.m.functions:
        for bb in fn.blocks:
            new_insts = []
            changed = False
            for inst in bb.instructions:
                si = inst.sync_info
                waits = list(si.on_wait) if si and si.on_wait else []
                upds = list(si.on_update) if si and si.on_update else []
                # compute engines can only encode plain sem-inc updates;
                # bacc nop-fusion sometimes folds a bulk 'sem-add-imm'
                # prebump into an adjacent compute op — move it back out
                can_bulk = type(inst).__name__ in (
                    "InstNoOp",
                    "InstDMACopy",
                    "InstEventSemaphore",
                    "InstDrain",
                )
                big_upds = [
                    u
                    for u in upds
                    if u.update_value > _MAX_UPD
                    or (u.update_mode != "sem-inc" and not can_bulk)
                ]
                if len(waits) > _MAX_WAITS or big_upds:
                    changed = True
                    head = (
                        waits[:-_MAX_WAITS] if len(waits) > _MAX_WAITS else []
                    )
                    tail = waits[-_MAX_WAITS:] if waits else []
                    # excess waits run on same-engine NOPs placed BEFORE
                    while head:
                        chunk, head = head[:_MAX_WAITS], head[_MAX_WAITS:]
                        new_insts.append(mk_nop(inst.engine, chunk, []))
                    # oversized sem bumps: keep <=cap on the inst, chain
                    # the remainder on NOPs AFTER it (observers can't fire
                    # earlier than the inst's completion either way)
                    keep_upds = []
                    after_nops = []
                    for u in upds:
                        v = u.update_value
                        if u not in big_upds:
                            keep_upds.append(u)
                            continue
                        keep_on_inst = can_bulk or u.update_mode == "sem-inc"
                        left = v
                        first = True
                        while left > 0:
                            step = min(left, _MAX_UPD)
                            left -= step
                            uu = mybir.SyncUpdate(
                                sync_type=u.sync_type,
                                id=u.id,
                                ant_name=u.ant_name,
                                update_mode=u.update_mode,
                                update_value=step,
                                update_reg=u.update_reg,
                            )
                            if first and keep_on_inst:
                                keep_upds.append(uu)
                            else:
                                after_nops.append(
                                    mk_nop(inst.engine, [], [uu])
                                )
                            first = False
                    inst.sync_info = mybir.SyncInfo(
                        on_wait=tail, on_update=keep_upds
                    )
                    new_insts.append(inst)
                    new_insts.extend(after_nops)
                else:
                    new_insts.append(inst)
            if changed:
                bb.instructions = new_insts
    return counter[0]


def build_program(bn_nonzero: bool, t_total: int = T):
    nw = t_total // W
    nc = bass.Bass(
        "TRN2",
        target_bir_lowering=False,
        debug=False,
        enable_asserts=False,
        num_devices=B,
    )

    xT = nc.dram_tensor("xT", [DC, P, t_total], F32, kind="ExternalInput")
    wih = nc.dram_tensor("wih", [DC, P, G], F32, kind="ExternalInput")
    whh = nc.dram_tensor("whh", [KC, P, G], MV_DT, kind="ExternalInput")
    bias = nc.dram_tensor("bias", [P, MC], F32, kind="ExternalInput")
    h0r = nc.dram_tensor("h0r", [P, HC], F32, kind="ExternalInput")
    identd = nc.dram_tensor("identd", [P, P], F32, kind="ExternalInput")
    if bn_nonzero:
        bnr = nc.dram_tensor("bnr", [P, HC], F32, kind="ExternalInput")
    ys = nc.dram_tensor("ys", [t_total, H], F32, kind="ExternalOutput")

    ET = mybir.EngineType
    AF = mybir.ActivationFunctionType

    with tile.TileContext(nc) as tc:
        with (
            tc.tile_pool(name="pers", bufs=1) as pers,
            tc.tile_pool(name="stp", bufs=3) as stp,
            tc.tile_pool(name="p_ghrz", bufs=2, space="PSUM") as p_ghrz,
            tc.tile_pool(name="p_ghn", bufs=2, space="PSUM") as p_ghn,
            tc.tile_pool(name="p_gx", bufs=2, space="PSUM") as p_gx,
            tc.tile_pool(name="p_tr", bufs=2, space="PSUM") as p_tr,
        ):
            # ---- persistent SBUF state ----
            whh_sb = []
            for k in range(KC):
                tk = pers.tile([P, G], MV_DT, tag=f"whh{k}")
                nc.sync.dma_start(tk[:], whh[k])
                whh_sb.append(tk)
            wih_sb = []
            for c in range(DC):
                tcw = pers.tile([P, G], F32, tag=f"wih{c}")
                nc.sync.dma_start(tcw[:], wih[c])
                wih_sb.append(tcw)
            bias_sb = pers.tile([P, MC], F32, tag="bias")
            nc.sync.dma_start(bias_sb[:], bias[:, :])
            if bn_nonzero:
                bn_sb = pers.tile([P, HC], F32, tag="bn")
                nc.sync.dma_start(bn_sb[:], bnr[:, :])
            hf32 = pers.tile([P, HC], F32, tag="hf32")
            nc.sync.dma_start(hf32[:], h0r[:, :])
            hmv = pers.tile([P, HC], MV_DT, tag="hmv")
            nc.vector.tensor_copy(hmv[:], hf32[:])
            # identity comes in as an input: building it on-device would
            # use GPSIMD, adding a 5th sync-wait to the For_i back-edge
            # drain (walrus caps waits per instruction at 4)
            ident = pers.tile([P, P], F32, tag="ident")
            nc.sync.dma_start(ident[:], identd[:, :])

            gx_sb = pers.tile([P, W * MC], F32, tag="gx")
            gx3 = gx_sb[:].rearrange("p (t c) -> p t c", c=MC)
            ys_acc = pers.tile([P, W * HC], F32, tag="ysacc")
            ys3 = ys_acc[:].rearrange("p (t c) -> p t c", c=HC)

            with tc.For_i(0, t_total, W, hint_engines=(ET.PE,)) as t0:
                # ---- load x.T slice for this window ----
                xtt = []
                for c in range(DC):
                    xc = stp.tile([P, W], F32, tag=f"xtt{c}")
                    nc.sync.dma_start(xc[:], xT[c, :, ds(t0, W)])
                    xtt.append(xc)
                # ---- gx = W_ih @ x.T (+ biases) for the window ----
                for j in range(MC):
                    gps = p_gx.tile([P, W], F32, tag="gps")
                    for c in range(DC):
                        nc.tensor.matmul(
                            gps[:],
                            wih_sb[c][:, j * P : (j + 1) * P],
                            xtt[c][:],
                            start=(c == 0),
                            stop=(c == DC - 1),
                        )
                    nc.scalar.activation(
                        gx3[:, :, j],
                        gps[:],
                        AF.Identity,
                        bias=bias_sb[:, j : j + 1],
                        scale=1.0,
                    )
                # ---- serial scan over the window ----
                for t in range(W):
                    gh_rz = p_ghrz.tile([P, 8], F32, tag="ghrz")
                    gh_n = p_ghn.tile([P, 4], F32, tag="ghn")
                    for m in range(MC):
                        out_ap = (
                            gh_rz[:, m : m + 1]
                            if m < 8
                            else gh_n[:, m - 8 : m - 7]
                        )
                        for k in range(KC):
                            nc.tensor.matmul(
                                out_ap,
                                whh_sb[k][:, m * P : (m + 1) * P],
                                hmv[:, k : k + 1],
                                start=(k == 0),
                                stop=(k == KC - 1),
                            )
                    a_rz = stp.tile([P, 8], F32, tag="arz")
                    nc.vector.tensor_add(a_rz[:], gh_rz[:], gx3[:, t, 0:8])
                    rz = stp.tile([P, 8], F32, tag="rz")
                    nc.scalar.activation(rz[:], a_rz[:], AF.Sigmoid)
                    zc = stp.tile([P, 4], F32, tag="zc")
                    nc.scalar.activation(zc[:], a_rz[:, 4:8], AF.Sigmoid, scale=-1.0)
                    if bn_nonzero:
                        ghn_b = stp.tile([P, 4], F32, tag="ghnb")
                        nc.vector.tensor_add(ghn_b[:], gh_n[:], bn_sb[:])
                        hn_ap = ghn_b[:]
                    else:
                        hn_ap = gh_n[:]
                    pp = stp.tile([P, 4], F32, tag="pp")
                    nc.vector.tensor_mul(pp[:], rz[:, 0:4], hn_ap)
                    uu = stp.tile([P, 4], F32, tag="uu")
                    nc.vector.tensor_add(uu[:], pp[:], gx3[:, t, 8:12])
                    nn = stp.tile([P, 4], F32, tag="nn")
                    nc.scalar.activation(nn[:], uu[:], AF.Tanh)
                    hprev = hf32[:] if t == 0 else ys3[:, t - 1, :]
                    ww = stp.tile([P, 4], F32, tag="ww")
                    nc.vector.tensor_mul(ww[:], rz[:, 4:8], hprev)
                    vv = stp.tile([P, 4], F32, tag="vv")
                    nc.vector.tensor_mul(vv[:], zc[:], nn[:])
                    nc.vector.tensor_add(ys3[:, t, :], ww[:], vv[:])
                    # state for next step's matvec (bf16 cast on DVE)
                    nc.vector.tensor_copy(hmv[:], ys3[:, t, :])
                # ---- carry fp32 state across windows ----
                nc.vector.tensor_copy(hf32[:], ys3[:, W - 1, :])
                # ---- transpose + write out ys window ----
                ysout = stp.tile([W, H], F32, tag="ysout")
                for c in range(HC):
                    trp = p_tr.tile([W, P], F32, tag="trp")
                    nc.tensor.transpose(trp[:], ys3[:, :, c], ident[:])
                    nc.scalar.copy(ysout[:, c * P : (c + 1) * P], trp[:])
                nc.sync.dma_start(ys[ds(t0, W), :], ysout[:])

    _legalize_waits(nc)
    return nc


def build_noop_program(bn_nonzero: bool, t_total: int = T):
    """Same external IO as build_program, near-zero device work.

    Used by test.py to subtract host<->device transfer + dispatch wall
    time from the full kernel's wall time (no NTFF profiling is
    available under this axon client, so device time is measured as a
    wall-clock difference at equal IO).
    """
    nc = bass.Bass(
        "TRN2",
        target_bir_lowering=False,
        debug=False,
        enable_asserts=False,
        num_devices=B,
    )
    xT = nc.dram_tensor("xT", [DC, P, t_total], F32, kind="ExternalInput")
    wih = nc.dram_tensor("wih", [DC, P, G], F32, kind="ExternalInput")
    whh = nc.dram_tensor("whh", [KC, P, G], MV_DT, kind="ExternalInput")
    bias = nc.dram_tensor("bias", [P, MC], F32, kind="ExternalInput")
    h0r = nc.dram_tensor("h0r", [P, HC], F32, kind="ExternalInput")
    identd = nc.dram_tensor("identd", [P, P], F32, kind="ExternalInput")
    if bn_nonzero:
        bnr = nc.dram_tensor("bnr", [P, HC], F32, kind="ExternalInput")
    ys = nc.dram_tensor("ys", [t_total, H], F32, kind="ExternalOutput")
    with tile.TileContext(nc) as tc:
        with tc.tile_pool(name="np_", bufs=1) as pool:
            t1 = pool.tile([P, P], F32, tag="t1")
            nc.sync.dma_start(t1[:], identd[:, :])
            nc.sync.dma_start(ys[0:P, 0:P], t1[:])
    _legalize_waits(nc)
    return nc


def _get_program(bn_nonzero: bool, t_total: int = T):
    key = (bn_nonzero, t_total, MATVEC_FP32)
    if key not in _prog_cache:
        _prog_cache[key] = build_program(bn_nonzero, t_total)
    return _prog_cache[key]


def _prep_shared(W_ih, W_hh, b_ih, b_hh):
    W_ih = np.asarray(W_ih, np.float32)
    W_hh = np.asarray(W_hh, np.float32)
    b_ih = np.asarray(b_ih, np.float32)
    b_hh = np.asarray(b_hh, np.float32)
    whh_in = np.ascontiguousarray(W_hh.T).astype(MV_NP).reshape(KC, P, G)
    wih_in = np.ascontiguousarray(W_ih.T).reshape(DC, P, G)
    # bias col j covers gate rows j*128..(j+1)*128; fold b_hh for r,z blocks
    bias_in = np.ascontiguousarray(b_ih.reshape(MC, P).T)
    bias_in[:, :8] += b_hh.reshape(MC, P).T[:, :8]
    bn = b_hh[2 * H :]
    bn_nonzero = bool(np.any(bn != 0.0))
    bn_in = np.ascontiguousarray(bn.reshape(HC, P).T) if bn_nonzero else None
    return whh_in, wih_in, bias_in, bn_nonzero, bn_in


def kernel(x, h0, W_ih, W_hh, b_ih, b_hh):
    x = np.asarray(x, np.float32)
    h0 = np.asarray(h0, np.float32)
    whh_in, wih_in, bias_in, bn_nonzero, bn_in = _prep_shared(
        W_ih, W_hh, b_ih, b_hh
    )
    nc = _get_program(bn_nonzero)

    in_maps = []
    for b in range(B):
        m = {
            "xT": np.ascontiguousarray(x[b].T).reshape(DC, P, T),
            "wih": wih_in,
            "whh": whh_in,
            "bias": bias_in,
            "h0r": np.ascontiguousarray(h0[b].reshape(HC, P).T),
            "identd": np.eye(P, dtype=np.float32),
        }
        if bn_nonzero:
            m["bnr"] = bn_in
        in_maps.append(m)

    res = bass_utils.run_bass_kernel_spmd(nc, in_maps, core_ids=list(range(B)))
    ys = np.stack([np.asarray(res.results[b]["ys"]) for b in range(B)])
    h_last = np.ascontiguousarray(ys[:, -1, :])
    return ys, h_last


# revision 22
# speedup vs baseline: 3.0691x; 3.0691x over previous
"""GRU sequence kernel (Cell2SeqWrapper) for Trainium2, 8 NeuronCores.

Problem: x:(8,4096,256) f32, h0:(8,512), W_ih:(1536,256), W_hh:(1536,512),
b_ih/b_hh:(1536,). Returns (ys:(8,4096,512), h_last:(8,512)) matching

    gx = einsum('btd,gd->btg', x, W_ih) + b_ih
    h' = (1-z) * n + z * h   (standard GRU cell, scan over T)

Sharding: data-parallel over batch: core b processes sequence b end-to-end
(weights replicated). Inside each core:
  - input projection gx computed on-device per 64-step window (TensorE)
  - the serial scan runs one GRU step at a time:
      * gh = W_hh @ h as 12x4 accumulated matvecs, W_hh stationary (bf16,
        fast-weight-load), h moving (bf16 cast of fp32 state)
      * gates on VectorE/ScalarE in fp32, state kept fp32
  - ys accumulated in SBUF, transposed via TensorE, DMA'd out per window
"""

import os
import sys

import numpy as np

sys.path.insert(0, "/opt/trn_rl_repo")

import ml_dtypes  # noqa: E402

import concourse.bass as bass  # noqa: E402
import concourse.mybir as mybir  # noqa: E402
import concourse.tile as tile  # noqa: E402
import concourse.tile_sem_assignment as _tsa  # noqa: E402
from concourse import bass_utils  # noqa: E402
from concourse.bass import ds  # noqa: E402

# Confine HW-DGE DMA completions to a single semaphore lane. With the
# default 8-lane round-robin, the For_i back-edge drain accumulates a
# sync-wait per lane plus one per engine and exceeds the wait-command
# limit in walrus codegen ("Too many sync wait commands").
# (8-lane HW-DGE round-robin kept; oversized wait lists are legalized below)

B, T, D, H = 8, 4096, 256, 512
G = 3 * H            # 1536 gate rows
P = 128              # partitions
KC = H // P          # 4 contraction chunks for W_hh
MC = G // P          # 12 output chunks
DC = D // P          # 2 contraction chunks for W_ih
HC = H // P          # 4 h chunks
W = 4                # scan window (steps per For_i iteration)
NW = T // W

F32 = mybir.dt.float32
BF16 = mybir.dt.bfloat16

# matvec dtype: bf16 (FWL, ~2x faster weight loads) unless overridden
MATVEC_FP32 = bool(int(os.environ.get("BASS_GRU_FP32", "0")))
MV_DT = F32 if MATVEC_FP32 else BF16
MV_NP = np.float32 if MATVEC_FP32 else ml_dtypes.bfloat16

_prog_cache = {}

# walrus's setupSyncWait rejects instructions carrying more than 3 sync
# waits ("Too many sync wait commands"). Tile occasionally emits more
# (loop back-edge drains, staggered-reset stage entries). Legalize by
# moving excess waits onto same-engine NOPs inserted just before the
# instruction — engines execute their queue in order, so the waits
# still complete before the original instruction runs.
_MAX_WAITS = 1


_MAX_UPD = 63


def _legalize_waits(nc):
    counter = [0]

    def mk_nop(engine, on_wait, on_update):
        counter[0] += 1
        nop = mybir.InstNoOp(name=f"I-legalsync-{counter[0]}", ins=[], outs=[])
        nop.engine = engine
        nop.sync_info = mybir.SyncInfo(on_wait=on_wait, on_update=on_update)
        nc.register_instruction(nop, overwrite=True)
        return nop

    for fn in nc.m.functions:
        for bb in fn.blocks:
            new_insts = []
            changed = False
            for inst in bb.instructions:
                si = inst.sync_info
                waits = list(si.on_wait) if si and si.on_wait else []
                upds = list(si.on_update) if si and si.on_update else []
                # compute engines can only encode plain sem-inc updates;
                # bacc nop-fusion sometimes folds a bulk 'sem-add-imm'
                # prebump into an adjacent compute op — move it back out
                can_bulk = type(inst).__name__ in (
                    "InstNoOp",
                    "InstDMACopy",
                    "InstEventSemaphore",
                    "InstDrain",
                )
                big_upds = [
                    u
                    for u in upds
                    if u.update_value > _MAX_UPD
                    or (u.update_mode != "sem-inc" and not can_bulk)
                ]
                if len(waits) > _MAX_WAITS or big_upds:
                    changed = True
                    head = (
                        waits[:-_MAX_WAITS] if len(waits) > _MAX_WAITS else []
                    )
                    tail = waits[-_MAX_WAITS:] if waits else []
                    # excess waits run on same-engine NOPs placed BEFORE
                    while head:
                        chunk, head = head[:_MAX_WAITS], head[_MAX_WAITS:]
                        new_insts.append(mk_nop(inst.engine, chunk, []))
                    # oversized sem bumps: keep <=cap on the inst, chain
                    # the remainder on NOPs AFTER it (observers can't fire
                    # earlier than the inst's completion either way)
                    keep_upds = []
                    after_nops = []
                    for u in upds:
                        v = u.update_value
                        if u not in big_upds:
                            keep_upds.append(u)
                            continue
                        keep_on_inst = can_bulk or u.update_mode == "sem-inc"
                        left = v
                        first = True
                        while left > 0:
                            step = min(left, _MAX_UPD)
                            left -= step
                            uu = mybir.SyncUpdate(
                                sync_type=u.sync_type,
                                id=u.id,
                                ant_name=u.ant_name,
                                update_mode=u.update_mode,
                                update_value=step,
                                update_reg=u.update_reg,
                            )
                            if first and keep_on_inst:
                                keep_upds.append(uu)
                            else:
                                after_nops.append(
                                    mk_nop(inst.engine, [], [uu])
                                )
                            first = False
                    inst.sync_info = mybir.SyncInfo(
                        on_wait=tail, on_update=keep_upds
                    )
                    new_insts.append(inst)
                    new_insts.extend(after_nops)
                else:
                    new_insts.append(inst)
            if changed:
                bb.instructions = new_insts
    return counter[0]


def build_program(bn_nonzero: bool, t_total: int = T):
    nw = t_total // W
    nc = bass.Bass(
        "TRN2",
        target_bir_lowering=False,
        debug=False,
        enable_asserts=False,
        num_devices=B,
    )

    xT = nc.dram_tensor("xT", [DC, P, t_total], F32, kind="ExternalInput")
    wih = nc.dram_tensor("wih", [DC, P, G], F32, kind="ExternalInput")
    whh = nc.dram_tensor("whh", [KC, P, G], MV_DT, kind="ExternalInput")
    bias = nc.dram_tensor("bias", [P, MC], F32, kind="ExternalInput")
    h0r = nc.dram_tensor("h0r", [P, HC], F32, kind="ExternalInput")
    identd = nc.dram_tensor("identd", [P, P], F32, kind="ExternalInput")
    if bn_nonzero:
        bnr = nc.dram_tensor("bnr", [P, HC], F32, kind="ExternalInput")
    ys = nc.dram_tensor("ys", [t_total, H], F32, kind="ExternalOutput")

    ET = mybir.EngineType
    AF = mybir.ActivationFunctionType

    with tile.TileContext(nc) as tc:
        with (
            tc.tile_pool(name="pers", bufs=1) as pers,
            tc.tile_pool(name="stp", bufs=3) as stp,
            tc.tile_pool(name="p_ghrz", bufs=2, space="PSUM") as p_ghrz,
            tc.tile_pool(name="p_ghn", bufs=2, space="PSUM") as p_ghn,
            tc.tile_pool(name="p_gx", bufs=2, space="PSUM") as p_gx,
            tc.tile_pool(name="p_tr", bufs=2, space="PSUM") as p_tr,
        ):
            # ---- persistent SBUF state ----
            whh_sb = []
            for k in range(KC):
                tk = pers.tile([P, G], MV_DT, tag=f"whh{k}")
                nc.sync.dma_start(tk[:], whh[k])
                whh_sb.append(tk)
            wih_sb = []
            for c in range(DC):
                tcw = pers.tile([P, G], F32, tag=f"wih{c}")
                nc.sync.dma_start(tcw[:], wih[c])
                wih_sb.append(tcw)
            bias_sb = pers.tile([P, MC], F32, tag="bias")
            nc.sync.dma_start(bias_sb[:], bias[:, :])
            if bn_nonzero:
                bn_sb = pers.tile([P, HC], F32, tag="bn")
                nc.sync.dma_start(bn_sb[:], bnr[:, :])
            hf32 = pers.tile([P, HC], F32, tag="hf32")
            nc.sync.dma_start(hf32[:], h0r[:, :])
            hmv = pers.tile([P, HC], MV_DT, tag="hmv")
            nc.vector.tensor_copy(hmv[:], hf32[:])
            # identity comes in as an input: building it on-device would
            # use GPSIMD, adding a 5th sync-wait to the For_i back-edge
            # drain (walrus caps waits per instruction at 4)
            ident = pers.tile([P, P], F32, tag="ident")
            nc.sync.dma_start(ident[:], identd[:, :])

            gx_sb = pers.tile([P, W * MC], F32, tag="gx")
            gx3 = gx_sb[:].rearrange("p (t c) -> p t c", c=MC)
            ys_acc = pers.tile([P, W * HC], F32, tag="ysacc")
            ys3 = ys_acc[:].rearrange("p (t c) -> p t c", c=HC)

            with tc.For_i(0, t_total, W, hint_engines=(ET.PE,)) as t0:
                # ---- load x.T slice for this window ----
                xtt = []
                for c in range(DC):
                    xc = stp.tile([P, W], F32, tag=f"xtt{c}")
                    nc.sync.dma_start(xc[:], xT[c, :, ds(t0, W)])
                    xtt.append(xc)
                # ---- gx = W_ih @ x.T (+ biases) for the window ----
                for j in range(MC):
                    gps = p_gx.tile([P, W], F32, tag="gps")
                    for c in range(DC):
                        nc.tensor.matmul(
                            gps[:],
                            wih_sb[c][:, j * P : (j + 1) * P],
                            xtt[c][:],
                            start=(c == 0),
                            stop=(c == DC - 1),
                        )
                    nc.scalar.activation(
                        gx3[:, :, j],
                        gps[:],
                        AF.Identity,
                        bias=bias_sb[:, j : j + 1],
                        scale=1.0,
                    )
                # ---- serial scan over the window ----
                for t in range(W):
                    gh_rz = p_ghrz.tile([P, 8], F32, tag="ghrz")
                    gh_n = p_ghn.tile([P, 4], F32, tag="ghn")
                    for m in range(MC):
                        out_ap = (
                            gh_rz[:, m : m + 1]
                            if m < 8
                            else gh_n[:, m - 8 : m - 7]
                        )
                        for k in range(KC):
                            nc.tensor.matmul(
                                out_ap,
                                whh_sb[k][:, m * P : (m + 1) * P],
                                hmv[:, k : k + 1],
                                start=(k == 0),
                                stop=(k == KC - 1),
                            )
                    a_rz = stp.tile([P, 8], F32, tag="arz")
                    nc.vector.tensor_add(a_rz[:], gh_rz[:], gx3[:, t, 0:8])
                    rz = stp.tile([P, 8], F32, tag="rz")
                    nc.scalar.activation(rz[:], a_rz[:], AF.Sigmoid)
                    zc = stp.tile([P, 4], F32, tag="zc")
                    nc.scalar.activation(zc[:], a_rz[:, 4:8], AF.Sigmoid, scale=-1.0)
                    if bn_nonzero:
                        ghn_b = stp.tile([P, 4], F32, tag="ghnb")
                        nc.vector.tensor_add(ghn_b[:], gh_n[:], bn_sb[:])
                        hn_ap = ghn_b[:]
                    else:
                        hn_ap = gh_n[:]
                    pp = stp.tile([P, 4], F32, tag="pp")
                    nc.vector.tensor_mul(pp[:], rz[:, 0:4], hn_ap)
                    uu = stp.tile([P, 4], F32, tag="uu")
                    nc.vector.tensor_add(uu[:], pp[:], gx3[:, t, 8:12])
                    nn = stp.tile([P, 4], F32, tag="nn")
                    nc.scalar.activation(nn[:], uu[:], AF.Tanh)
                    hprev = hf32[:] if t == 0 else ys3[:, t - 1, :]
                    ww = stp.tile([P, 4], F32, tag="ww")
                    nc.vector.tensor_mul(ww[:], rz[:, 4:8], hprev)
                    vv = stp.tile([P, 4], F32, tag="vv")
                    nc.vector.tensor_mul(vv[:], zc[:], nn[:])
                    nc.vector.tensor_add(ys3[:, t, :], ww[:], vv[:])
                    # state for next step's matvec (bf16 cast on DVE)
                    nc.vector.tensor_copy(hmv[:], ys3[:, t, :])
                # ---- carry fp32 state across windows ----
                nc.vector.tensor_copy(hf32[:], ys3[:, W - 1, :])
                # ---- transpose + write out ys window ----
                ysout = stp.tile([W, H], F32, tag="ysout")
                for c in range(HC):
                    trp = p_tr.tile([W, P], F32, tag="trp")
                    nc.tensor.transpose(trp[:], ys3[:, :, c], ident[:])
                    nc.scalar.copy(ysout[:, c * P : (c + 1) * P], trp[:])
                nc.sync.dma_start(ys[ds(t0, W), :], ysout[:])

    _legalize_waits(nc)
    return nc


def build_noop_program(bn_nonzero: bool, t_total: int = T):
    """Same external IO as build_program, near-zero device work.

    Used by test.py to subtract host<->device transfer + dispatch wall
    time from the full kernel's wall time (no NTFF profiling is
    available under this axon client, so device time is measured as a
    wall-clock difference at equal IO).
    """
    nc = bass.Bass(
        "TRN2",
        target_bir_lowering=False,
        debug=False,
        enable_asserts=False,
        num_devices=B,
    )
    xT = nc.dram_tensor("xT", [DC, P, t_total], F32, kind="ExternalInput")
    wih = nc.dram_tensor("wih", [DC, P, G], F32, kind="ExternalInput")
    whh = nc.dram_tensor("whh", [KC, P, G], MV_DT, kind="ExternalInput")
    bias = nc.dram_tensor("bias", [P, MC], F32, kind="ExternalInput")
    h0r = nc.dram_tensor("h0r", [P, HC], F32, kind="ExternalInput")
    identd = nc.dram_tensor("identd", [P, P], F32, kind="ExternalInput")
    if bn_nonzero:
        bnr = nc.dram_tensor("bnr", [P, HC], F32, kind="ExternalInput")
    ys = nc.dram_tensor("ys", [t_total, H], F32, kind="ExternalOutput")
    with tile.TileContext(nc) as tc:
        with tc.tile_pool(name="np_", bufs=1) as pool:
            t1 = pool.tile([P, P], F32, tag="t1")
            nc.sync.dma_start(t1[:], identd[:, :])
            nc.sync.dma_start(ys[0:P, 0:P], t1[:])
    _legalize_waits(nc)
    return nc


def _get_program(bn_nonzero: bool, t_total: int = T):
    key = (bn_nonzero, t_total, MATVEC_FP32)
    if key not in _prog_cache:
        _prog_cache[key] = build_program(bn_nonzero, t_total)
    return _prog_cache[key]


def _prep_shared(W_ih, W_hh, b_ih, b_hh):
    W_ih = np.asarray(W_ih, np.float32)
    W_hh = np.asarray(W_hh, np.float32)
    b_ih = np.asarray(b_ih, np.float32)
    b_hh = np.asarray(b_hh, np.float32)
    whh_in = np.ascontiguousarray(W_hh.T).astype(MV_NP).reshape(KC, P, G)
    wih_in = np.ascontiguousarray(W_ih.T).reshape(DC, P, G)
    # bias col j covers gate rows j*128..(j+1)*128; fold b_hh for r,z blocks
    bias_in = np.ascontiguousarray(b_ih.reshape(MC, P).T)
    bias_in[:, :8] += b_hh.reshape(MC, P).T[:, :8]
    bn = b_hh[2 * H :]
    bn_nonzero = bool(np.any(bn != 0.0))
    bn_in = np.ascontiguousarray(bn.reshape(HC, P).T) if bn_nonzero else None
    return whh_in, wih_in, bias_in, bn_nonzero, bn_in


def kernel(x, h0, W_ih, W_hh, b_ih, b_hh):
    x = np.asarray(x, np.float32)
    h0 = np.asarray(h0, np.float32)
    whh_in, wih_in, bias_in, bn_nonzero, bn_in = _prep_shared(
        W_ih, W_hh, b_ih, b_hh
    )
    nc = _get_program(bn_nonzero)

    in_maps = []
    for b in range(B):
        m = {
            "xT": np.ascontiguousarray(x[b].T).reshape(DC, P, T),
            "wih": wih_in,
            "whh": whh_in,
            "bias": bias_in,
            "h0r": np.ascontiguousarray(h0[b].reshape(HC, P).T),
            "identd": np.eye(P, dtype=np.float32),
        }
        if bn_nonzero:
            m["bnr"] = bn_in
        in_maps.append(m)

    res = bass_utils.run_bass_kernel_spmd(nc, in_maps, core_ids=list(range(B)))
    ys = np.stack([np.asarray(res.results[b]["ys"]) for b in range(B)])
    h_last = np.ascontiguousarray(ys[:, -1, :])
    return ys, h_last
